# revision 12
# baseline (speedup 1.0000x reference)
"""AttnGRU Trainium2 kernel: 8-way data-parallel over the node dimension.

Per node n (E=32 edges, H=256, A=64):
  q[n]   = (temp[n] @ Wt.T + bt) @ (Ws * temperature)     # bs cancels in softmax
  attn_e = <spat[n,e,:], q[n]>;  w = softmax_e(attn)      # exp/sum, no max-sub (|logit|<~60)
  Hagg   = sum_e w[e] spat[n,e,:]
  x_e=relu(xy@W_xy.T); H_e=relu([temp,Hagg]@W_he.T); GRU(cat=[x_e,H_e], prev) -> hidden
  predict = hidden @ W_pred.T + b_pred

Mapping: spat streams edge-major ([4 nodes x 32 edges] partitions x [256h | 2 ones] cols,
float32r for TF32-rate matmuls). Attention dots = DVE scalar_tensor_tensor fused
multiply+row-reduce vs PE-replicated q. Weighted aggregation = PE matmuls with
block-diagonal scattered softmax numerators (exp), PSUM-accumulated 8 K-chunks per
32-node output block; ones-columns give the normalizer in the same matmul. Nodes within
each 32-block are permuted (m = 8*nl + j <-> node 4*j + nl) so the exp-weight scatter is
four plain strided DMAs per block. GRU/MLP chain = feature-major fp32r matmuls at N=512.
"""
import sys
sys.path.insert(0, '/opt/trn_rl_repo')
from contextlib import ExitStack

import numpy as np

import concourse.bass as bass
import concourse.tile as tile
from concourse import bacc, mybir, masks

f32 = mybir.dt.float32
f32r = mybir.dt.float32r
AF = mybir.ActivationFunctionType
OP = mybir.AluOpType

N_TOTAL, E, H, A = 16384, 32, 256, 64
OUT_DIM = 5
N_CORES = 8
TEMPERATURE = E / np.sqrt(A)

SUP = 512              # nodes per super-tile
NBLK = SUP // 32       # 16 32-node blocks per super-tile

_CACHE = {}


def build_kernel(n_core):
    nsup = n_core // SUP
    nc = bacc.Bacc("TRN2", target_bir_lowering=False, debug=False, num_devices=1)

    def din(name, shape, dt=f32r):
        return nc.dram_tensor(name, shape, dt, kind="ExternalInput").ap()

    spat = din("spat", [n_core * E, H])
    temp = din("temp", [n_core, H])
    prev = din("prev", [n_core, H])
    xyT = din("xyT", [2, n_core])
    WtT = din("WtT", [H, A])
    bt_d = din("bt", [A, 1], f32)
    Wsx = din("Wsx", [A, H])
    WxyT = din("WxyT", [2, H])
    WheT = din("WheT", [2 * H, H])
    WihT = din("WihT", [2 * H, 3 * H])
    WhhT = din("WhhT", [H, 3 * H])
    bias_g = din("bias_g", [3 * H, 1], f32)
    WpT = din("WpT", [H, 6])
    bp_d = din("bp", [6, 1], f32)
    Rmat = din("R", [32, 8 * 128])

    hid_o = nc.dram_tensor("hid", [n_core, H], f32, kind="ExternalOutput").ap()
    prd_o = nc.dram_tensor("prd", [n_core, 6], f32, kind="ExternalOutput").ap()
    import os
    DBG = os.environ.get("KDBG") == "1"
    if DBG:
        dbg_te = nc.dram_tensor("dbg_te", [A, SUP], f32, kind="ExternalOutput").ap()
        dbg_q = nc.dram_tensor("dbg_q", [32, NBLK * H], f32, kind="ExternalOutput").ap()
        dbg_at = nc.dram_tensor("dbg_at", [128, 8], f32, kind="ExternalOutput").ap()
        dbg_hg = nc.dram_tensor("dbg_hg", [32, H], f32, kind="ExternalOutput").ap()
        dbg_tT = nc.dram_tensor("dbg_tT", [128, SUP], f32, kind="ExternalOutput").ap()
        dbg_ti = nc.dram_tensor("dbg_ti", [128, H], f32, kind="ExternalOutput").ap()

    # permuted views of node-major DRAM tensors: within each 32-node block,
    # SBUF partition m = 8*nl + j holds DRAM row 4*j + nl.
    def perm_view(ap_2d):  # [rows, width] -> [blk, nl, j, width]
        return ap_2d.rearrange("(b j nl) w -> b nl j w", j=8, nl=4)

    temp_pv = perm_view(temp)
    prev_pv = perm_view(prev)
    hid_pv = perm_view(hid_o)
    prd_pv = perm_view(prd_o)
    spat_r = spat.rearrange("(g p) h -> g p h", p=128)

    with tile.TileContext(nc) as tc, ExitStack() as ctx:
        wp = ctx.enter_context(tc.tile_pool(name="weights", bufs=1))
        sb = ctx.enter_context(tc.tile_pool(name="sb", bufs=2))
        sb3 = ctx.enter_context(tc.tile_pool(name="sb3", bufs=3))
        pp = ctx.enter_context(tc.tile_pool(name="pp", bufs=2, space="PSUM"))
        ppt = ctx.enter_context(tc.tile_pool(name="ppt", bufs=2, space="PSUM"))
        ppc = ctx.enter_context(tc.tile_pool(name="ppc", bufs=2, space="PSUM"))
        ppu = ctx.enter_context(tc.tile_pool(name="ppu", bufs=2, space="PSUM"))

        wtT = wp.tile([128, 2 * A], f32r)
        nc.sync.dma_start(wtT[:].rearrange("p (k c) -> p k c", k=2), WtT.rearrange("(k p) c -> p k c", p=128))
        bt = wp.tile([A, 1], f32); nc.sync.dma_start(bt[:], bt_d[:])
        wsx = wp.tile([A, H], f32r); nc.sync.dma_start(wsx[:], Wsx[:])
        wxyT = wp.tile([2, H], f32r); nc.sync.dma_start(wxyT[:], WxyT[:])
        wheT = wp.tile([128, 4 * H], f32r)
        nc.sync.dma_start(wheT[:].rearrange("p (k c) -> p k c", k=4), WheT.rearrange("(k p) c -> p k c", p=128))
        wihT = wp.tile([128, 4 * 3 * H], f32r)
        nc.sync.dma_start(wihT[:].rearrange("p (k c) -> p k c", k=4), WihT.rearrange("(k p) c -> p k c", p=128))
        whhT = wp.tile([128, 2 * 3 * H], f32r)
        nc.sync.dma_start(whhT[:].rearrange("p (k c) -> p k c", k=2), WhhT.rearrange("(k p) c -> p k c", p=128))
        biasg = wp.tile([128, 6], f32)
        nc.sync.dma_start(biasg[:].rearrange("p (k c) -> p k c", k=6), bias_g.rearrange("(k p) c -> p k c", p=128))
        wpT = wp.tile([128, 12], f32r)
        nc.sync.dma_start(wpT[:].rearrange("p (k c) -> p k c", k=2), WpT.rearrange("(k p) c -> p k c", p=128))
        bp = wp.tile([6, 1], f32); nc.sync.dma_start(bp[:], bp_d[:])
        rmat = wp.tile([32, 8 * 128], f32r); nc.sync.dma_start(rmat[:], Rmat[:])
        xyt_all = wp.tile([2, n_core], f32r); nc.sync.dma_start(xyt_all[:], xyT[:])
        identf = wp.tile([128, 128], f32); masks.make_identity(nc, identf[:])
        ident = wp.tile([128, 128], f32r); nc.vector.tensor_copy(ident[:], identf[:])

        NSP = 24
        spats = [wp.tile([128, H + 2], f32r, name=f"spat{i}") for i in range(NSP)]
        for i in range(NSP):
            nc.vector.memset(spats[i][:, H:H + 2].bitcast(f32), 1.0)
        bd_all = wp.tile([128, 292], f32r)
        nc.vector.memset(bd_all[:].bitcast(f32), 0.0)

        def trans128(dst_tile, dst_col, src_ap):
            pt = ppt.tile([128, 128], f32, tag="ptr", name="pt_tr")
            nc.tensor.matmul(pt[:], src_ap.bitcast(f32), identf[:], is_transpose=True, start=True, stop=True)
            nc.scalar.copy(dst_tile[:, dst_col:dst_col + 128], pt[:])

        for s in range(nsup):
            n0 = s * SUP
            b0 = s * NBLK
            tempTc = [sb.tile([128, SUP], f32r, name=f"tempT{c}", tag=f"tempT{c}") for c in range(2)]
            prevTc = [sb.tile([128, SUP], f32r, name=f"prevT{c}", tag=f"prevT{c}") for c in range(2)]
            for sub in range(4):
                tin = sb.tile([128, H], f32r, tag="tin", name="tin")
                pin = sb.tile([128, H], f32r, tag="pin", name="pin")
                nc.sync.dma_start(tin[:], temp[n0 + sub * 128:n0 + (sub + 1) * 128, :])
                nc.sync.dma_start(pin[:], prev[n0 + sub * 128:n0 + (sub + 1) * 128, :])
                if DBG and s == 0 and sub == 0:
                    nc.sync.dma_start(dbg_ti[:], tin[:].bitcast(f32))
                for c in range(2):
                    trans128(tempTc[c], sub * 128, tin[:, c * 128:(c + 1) * 128])
                    trans128(prevTc[c], sub * 128, pin[:, c * 128:(c + 1) * 128])

            # te.T = Wt @ temp.T + bt : [64, 512]
            pte = ppc.tile([A, SUP], f32, tag="pchain", name="pte")
            for c in range(2):
                nc.tensor.matmul(pte[:], wtT[:, c * A:(c + 1) * A], tempTc[c][:],
                                 start=(c == 0), stop=(c == 1))
            te_sb = sb.tile([A, SUP], f32r, tag="te", name="te_sb")
            nc.scalar.activation(te_sb[:], pte[:], AF.Identity, bias=bt[:], scale=1.0)
            if DBG and s == 0:
                nc.sync.dma_start(dbg_te[:], te_sb[:].bitcast(f32))
                nc.sync.dma_start(dbg_tT[:], tempTc[0][:].bitcast(f32))

            # q nodes-major [32, 256] per block -> q_sb2 [32, 16*256]
            q_sb2 = sb.tile([32, NBLK * H], f32r, tag="qsb2", name="q_sb2")
            for cb in range(NBLK):
                pq = ppc.tile([32, H], f32, tag="pchain", name="pq")
                nc.tensor.matmul(pq[:], te_sb[:, cb * 32:(cb + 1) * 32], wsx[:],
                                 start=True, stop=True)
                nc.scalar.copy(q_sb2[:, cb * H:(cb + 1) * H], pq[:])
            if DBG and s == 0:
                nc.sync.dma_start(dbg_q[:], q_sb2[:].bitcast(f32))

            # attention + aggregation
            haggTc = [sb.tile([128, SUP], f32r, name=f"haggT{c}", tag=f"haggT{c}") for c in range(2)]
            for cb in range(NBLK):
                attn = sb3.tile([128, 8], f32, tag="attn", name="attn")
                ew = sb3.tile([128, 8], f32, tag="ew", name="ew")
                uacc = ppu.tile([32, H + 2], f32, tag="uacc", name="uacc")
                stl = []
                for j in range(8):
                    g = (b0 + cb) * 8 + j
                    sp = spats[g % NSP]
                    stl.append(sp)
                    nc.sync.dma_start(sp[:, 0:H], spat_r[g])
                    pqr = pp.tile([128, H], f32, tag="pqr", name="pqr")
                    nc.tensor.matmul(pqr[:], rmat[:, j * 128:(j + 1) * 128],
                                     q_sb2[:, cb * H:(cb + 1) * H], start=True, stop=True)
                    scr = sb3.tile([128, H], f32, tag="scr", name="scr")
                    nc.vector.scalar_tensor_tensor(out=scr[:], in0=sp[:, 0:H].bitcast(f32),
                                                   scalar=1.0, in1=pqr[:],
                                                   op0=OP.mult, op1=OP.mult,
                                                   accum_out=attn[:, j:j + 1])
                if DBG and s == 0 and cb == 0:
                    nc.sync.dma_start(dbg_at[:], attn[:])
                nc.scalar.activation(ew[:], attn[:], AF.Exp)
                # scatter: bd_all[32nl+e, 33j+8nl] = ew[32nl+e, j]
                for nl in range(4):
                    dst = bd_all[nl * 32:(nl + 1) * 32, nl:nl + 288]
                    dstv = dst.rearrange("p (j r) -> p j r", j=8)[:, :, 0:1]
                    nc.sync.dma_start(dstv, ew[nl * 32:(nl + 1) * 32, :]
                                      .rearrange("p (j o) -> p j o", o=1).bitcast(f32r))
                for j in range(8):
                    nc.tensor.matmul(uacc[:], bd_all[:, 32 * j:32 * j + 32],
                                     stl[j][:], start=(j == 0), stop=(j == 7))
                rz = sb3.tile([32, 1], f32, tag="rz", name="rz")
                nc.vector.reciprocal(rz[:], uacc[:, H:H + 1])
                hagg = sb3.tile([32, H], f32, tag="hagg", name="hagg")
                nc.scalar.activation(hagg[:], uacc[:, 0:H], AF.Copy, bias=0.0, scale=rz[:])
                if DBG and s == 0 and cb == 0:
                    nc.sync.dma_start(dbg_hg[:], hagg[:].bitcast(f32))
                for c in range(2):
                    ph = ppt.tile([128, 32], f32, tag="ptr", name="ph")
                    nc.tensor.matmul(ph[:], hagg[:, c * 128:(c + 1) * 128], identf[0:32, 0:32], is_transpose=True, start=True, stop=True)
                    nc.scalar.copy(haggTc[c][:, cb * 32:(cb + 1) * 32], ph[:])

            # x_eT = relu(Wxy @ xyT)
            xeTc = [sb.tile([128, SUP], f32r, name=f"xeT{c}", tag=f"xeT{c}") for c in range(2)]
            for c in range(2):
                px = ppc.tile([128, SUP], f32, tag="pchain", name="px")
                nc.tensor.matmul(px[:], wxyT[:, c * 128:(c + 1) * 128],
                                 xyt_all[:, n0:n0 + SUP], start=True, stop=True)
                nc.scalar.activation(xeTc[c][:], px[:], AF.Relu)

            # H_eT = relu(Whe @ catT), cat = [temp, Hagg]
            heTc = [sb.tile([128, SUP], f32r, name=f"heT{c}", tag=f"heT{c}") for c in range(2)]
            cat1 = tempTc + haggTc
            for m in range(2):
                ph2 = ppc.tile([128, SUP], f32, tag="pchain", name="ph2")
                for k in range(4):
                    nc.tensor.matmul(ph2[:], wheT[:, k * H + m * 128:k * H + (m + 1) * 128],
                                     cat1[k][:], start=(k == 0), stop=(k == 3))
                nc.scalar.activation(heTc[m][:], ph2[:], AF.Relu)

            cat2 = xeTc + heTc

            def gate_psum(m):
                pg = ppc.tile([128, SUP], f32, tag="pchain", name=f"pg{m}")
                for k in range(4):
                    nc.tensor.matmul(pg[:], wihT[:, k * 3 * H + m * 128:k * 3 * H + (m + 1) * 128],
                                     cat2[k][:], start=(k == 0), stop=False)
                return pg

            def gate_hh(pg, m, rhs):
                for k in range(2):
                    nc.tensor.matmul(pg[:], whhT[:, k * 3 * H + m * 128:k * 3 * H + (m + 1) * 128],
                                     rhs[k][:], start=False, stop=(k == 1))

            zc, rc = [], []
            for m in range(4):
                pg = gate_psum(m); gate_hh(pg, m, prevTc)
                gsb = sb.tile([128, SUP], f32r, name=f"g{m}", tag=f"g{m}")
                nc.scalar.activation(gsb[:], pg[:], AF.Sigmoid,
                                     bias=biasg[:, m:m + 1], scale=1.0)
                (zc if m < 2 else rc).append(gsb)
            rpc = []
            for c in range(2):
                rp = sb.tile([128, SUP], f32r, name=f"rp{c}", tag=f"rp{c}")
                nc.vector.tensor_mul(rp[:], rc[c][:].bitcast(f32),
                                     prevTc[c][:].bitcast(f32))
                rpc.append(rp)
            nn_c = []
            for i, m in enumerate([4, 5]):
                pg = gate_psum(m); gate_hh(pg, m, rpc)
                gsb = sb.tile([128, SUP], f32, name=f"g{m}", tag=f"g{m}")
                nc.scalar.activation(gsb[:], pg[:], AF.Relu,
                                     bias=biasg[:, m:m + 1], scale=1.0)
                nn_c.append(gsb)
            hidTc = []
            for c in range(2):
                d = sb.tile([128, SUP], f32, name=f"d{c}", tag=f"d{c}")
                nc.vector.tensor_tensor(d[:], prevTc[c][:].bitcast(f32), nn_c[c][:], op=OP.subtract)
                nc.vector.tensor_mul(d[:], d[:], zc[c][:].bitcast(f32))
                hT = sb.tile([128, SUP], f32r, name=f"hT{c}", tag=f"hT{c}")
                nc.vector.tensor_add(hT[:], d[:], nn_c[c][:])
                hidTc.append(hT)

            ppd = ppc.tile([6, SUP], f32, tag="pchain", name="ppd")
            for k in range(2):
                nc.tensor.matmul(ppd[:], wpT[:, k * 6:(k + 1) * 6], hidTc[k][:],
                                 start=(k == 0), stop=(k == 1))
            prd_sb = sb.tile([6, SUP], f32, tag="prd", name="prd_sb")
            nc.scalar.activation(prd_sb[:], ppd[:], AF.Identity, bias=bp[:], scale=1.0)
            for sub in range(4):
                pp6 = ppt.tile([128, 6], f32, tag="ptr", name="pp6")
                nc.tensor.matmul(pp6[:], prd_sb[:, sub * 128:(sub + 1) * 128], identf[0:6, 0:6], is_transpose=True, start=True, stop=True)
                p6 = sb3.tile([128, 6], f32, tag="p6", name="p6")
                nc.scalar.copy(p6[:], pp6[:])
                nc.sync.dma_start(prd_o[n0 + sub * 128:n0 + (sub + 1) * 128, :], p6[:])
                hout = sb.tile([128, H], f32, tag="hout", name="hout")
                for c in range(2):
                    pt = ppt.tile([128, 128], f32, tag="ptr", name="pt_ho")
                    nc.tensor.matmul(pt[:], hidTc[c][:, sub * 128:(sub + 1) * 128].bitcast(f32), identf[:], is_transpose=True, start=True, stop=True)
                    nc.scalar.copy(hout[:, c * 128:(c + 1) * 128], pt[:])
                nc.sync.dma_start(hid_o[n0 + sub * 128:n0 + (sub + 1) * 128, :], hout[:])

    nc.compile()
    return nc


def host_prep(inputs, n_core):
    xy = np.ascontiguousarray(np.asarray(inputs["xy"], np.float32))
    temp = np.ascontiguousarray(np.asarray(inputs["temp_hidden"], np.float32))
    spat = np.ascontiguousarray(np.asarray(inputs["spat_hidden"], np.float32))
    prev = np.ascontiguousarray(np.asarray(inputs["prev_hidden"], np.float32))
    Wt = np.asarray(inputs["Wt"], np.float32); bt = np.asarray(inputs["bt"], np.float32)
    Ws = np.asarray(inputs["Ws"], np.float32)
    W_xy = np.asarray(inputs["W_xy"], np.float32)
    W_he = np.asarray(inputs["W_he"], np.float32)
    w_ih = np.asarray(inputs["weight_ih"], np.float32)
    w_hh = np.asarray(inputs["weight_hh"], np.float32)
    b_g = np.asarray(inputs["bias_g"], np.float32)
    W_pred = np.asarray(inputs["W_pred"], np.float32)
    b_pred = np.asarray(inputs["b_pred"], np.float32)

    # R_j [32, 128]: row k = q-row (node slot), col p = (nl, e): 1 iff k == 8*nl + j
    R = np.zeros((32, 8, 128), np.float32)
    for j in range(8):
        for p in range(128):
            R[4 * j + p // 32, j, p] = 1.0
    WpT = np.zeros((H, 6), np.float32); WpT[:, :OUT_DIM] = W_pred.T
    bp = np.zeros((6, 1), np.float32); bp[:OUT_DIM, 0] = b_pred
    wih_all = np.concatenate([w_ih[0], w_ih[1], w_ih[2]], axis=0)
    whh_all = np.concatenate([w_hh[0], w_hh[1], w_hh[2]], axis=0)
    xyTp = np.ascontiguousarray(xy.T)

    shared = {
        "WtT": Wt.T.copy(), "bt": bt.reshape(A, 1).copy(),
        "Wsx": (Ws * np.float32(TEMPERATURE)).copy(),
        "WxyT": W_xy.T.copy(), "WheT": W_he.T.copy(),
        "WihT": wih_all.T.copy(), "WhhT": whh_all.T.copy(),
        "bias_g": b_g.reshape(3 * H, 1).copy(),
        "WpT": WpT, "bp": bp, "R": R.reshape(32, 8 * 128).copy(),
    }
    in_maps = []
    for c in range(N_TOTAL // n_core):
        sl = slice(c * n_core, (c + 1) * n_core)
        m = dict(shared)
        m["spat"] = spat[sl].reshape(n_core * E, H)
        m["temp"] = temp[sl]
        m["prev"] = prev[sl]
        m["xyT"] = xyTp[:, c * n_core:(c + 1) * n_core].copy()
        in_maps.append(m)
    return in_maps


def kernel(**inputs):
    from concourse.bass_utils import run_bass_kernel_spmd
    npc = N_TOTAL // N_CORES
    if "nc" not in _CACHE:
        _CACHE["nc"] = build_kernel(npc)
    nc = _CACHE["nc"]
    in_maps = host_prep(inputs, npc)
    res = run_bass_kernel_spmd(nc, in_maps, core_ids=list(range(N_CORES)))
    predict = np.concatenate([res.results[c]["prd"][:, :OUT_DIM] for c in range(N_CORES)], axis=0)
    hidden = np.concatenate([res.results[c]["hid"] for c in range(N_CORES)], axis=0)
    return predict, hidden
